# revision 5
# baseline (speedup 1.0000x reference)
"""GAT encoder (2x GATConv + ELU + global mean pool) on 8 Trainium2 NeuronCores.

Self-contained: kernel(**inputs) takes the FULL inputs (as produced by the
problem's setup_inputs), shards the graph across 8 cores, compiles + runs a
Bass/Tile SPMD kernel via run_bass_kernel_spmd, and returns the FULL [64, 128]
output.

Distribution strategy (dst-major):
- Nodes are partitioned contiguously across cores, then each core's nodes are
  sorted by a 2D shell-angle order on (degA, degB), where A/B = source node in
  the low/high half of the relabeled global id space (keeps dma_gather's int16
  indices in range). Sorted nodes form blocks of 128 partitions.
- Each block's in-edges occupy slots along the free dim: A-slots gathered from
  table half A, B-slots from half B. Block widths are uniform across cores so
  one SPMD program serves all 8. Pad slots are masked additively before exp.
- Per layer: each core computes xl = x_own @ W_ext locally (W_ext carries
  folded att_src/att_dst columns), AllGathers the xl table, then runs the
  per-block pipeline (dma_gather of source rows; a_s via DVE mul+reduce;
  s = a_s + a_d + mask; lrelu/exp on ACT with accumulated denominators;
  msg = g * ex; slot-reduce; divide; bias; ELU).
- Mean pool: per-block matmul with a host-built one-hot into PSUM; the host
  sums the 8 partial [G, OUT] results and divides by graph counts.
"""
import sys

for _p in ("/opt/trn_rl_repo", "/root/.axon_site/_ro/trn_rl_repo"):
    if _p not in sys.path:
        sys.path.insert(0, _p)

from dataclasses import dataclass, field

import numpy as np

import concourse.bacc as bacc
import concourse.mybir as mybir
import concourse.tile as tile
from concourse import library_config
from concourse.bass_utils import run_bass_kernel_spmd

AF = mybir.ActivationFunctionType
OP = mybir.AluOpType
F32 = mybir.dt.float32
I16 = mybir.dt.int16

# problem constants (hardcoded per contract)
N_NODES = 50000
N_CORES = 8
IN_CH = 128
HID = 32
HEADS = 2
OUT_CH = 128
N_GRAPHS = 64
NEG_SLOPE = 0.2

NPC = N_NODES // N_CORES          # 6250 nodes per core
NB = (NPC + 127) // 128           # 49 blocks per core
NPP = NB * 128                    # 6272 padded
NG = N_CORES * NPP                # 50176 global padded ids
HG = NG // 2                      # 25088 (<= int16 max)
D1 = HEADS * HID                  # 64
D2 = OUT_CH                       # 128


@dataclass
class Prep:
    ja: list
    jb: list
    idxa_w: list = field(default_factory=list)
    idxb_w: list = field(default_factory=list)
    mask_u: list = field(default_factory=list)
    onehot: list = field(default_factory=list)
    counts: np.ndarray | None = None
    perms: np.ndarray | None = None


def _wrap_idxs(idxs: np.ndarray) -> np.ndarray:
    S = len(idxs)
    cols = S // 16
    a = idxs.astype(np.int16).reshape(cols, 16).T
    return np.tile(a, (8, 1)).copy()


def preprocess(edge_index: np.ndarray, batch: np.ndarray) -> Prep:
    src = np.concatenate([edge_index[0], np.arange(N_NODES, dtype=np.int64)])
    dst = np.concatenate([edge_index[1], np.arange(N_NODES, dtype=np.int64)])

    core_of = dst // NPC
    src_half_a = (src // NPC) < (N_CORES // 2)

    degA = np.zeros((N_CORES, NPC), dtype=np.int64)
    degB = np.zeros((N_CORES, NPC), dtype=np.int64)
    loc = dst - core_of * NPC
    np.add.at(degA, (core_of, loc), src_half_a)
    np.add.at(degB, (core_of, loc), ~src_half_a)

    perms = []
    for k in range(N_CORES):
        m = np.maximum(degA[k], degB[k])
        perms.append(np.lexsort((-(degA[k] - degB[k]), -m)))
    perms = np.stack(perms)

    pos = np.empty((N_CORES, NPC), dtype=np.int64)
    for k in range(N_CORES):
        pos[k, perms[k]] = np.arange(NPC)

    gid = np.empty(N_NODES, dtype=np.int64)
    for k in range(N_CORES):
        gid[k * NPC : (k + 1) * NPC] = k * NPP + pos[k]

    dA_pad = np.zeros((N_CORES, NPP), dtype=np.int64)
    dB_pad = np.zeros((N_CORES, NPP), dtype=np.int64)
    dA_pad[:, :NPC] = np.take_along_axis(degA, perms, axis=1)
    dB_pad[:, :NPC] = np.take_along_axis(degB, perms, axis=1)
    ja = np.maximum(dA_pad.reshape(N_CORES, NB, 128).max(axis=(0, 2)), 1)
    jb = np.maximum(dB_pad.reshape(N_CORES, NB, 128).max(axis=(0, 2)), 1)

    prep = Prep(ja=[int(v) for v in ja], jb=[int(v) for v in jb])
    prep.perms = perms

    e_core = core_of
    e_pos = pos[e_core, loc]
    e_gsrc = gid[src]
    key = (e_core * NPP + e_pos) * 2 + (~src_half_a)
    order_e = np.argsort(key, kind="stable")
    ksorted = key[order_e]
    firsts = np.r_[0, np.flatnonzero(np.diff(ksorted)) + 1]
    startrep = np.repeat(firsts, np.diff(np.r_[firsts, len(ksorted)]))
    jwithin = np.arange(len(ksorted)) - startrep
    e_j = np.empty_like(jwithin)
    e_j[order_e] = jwithin

    SA = int(ja.sum()) * 128
    SB = int(jb.sum()) * 128
    offA = np.r_[0, np.cumsum(ja)[:-1]] * 128
    offB = np.r_[0, np.cumsum(jb)[:-1]] * 128
    JTOT = int(ja.sum() + jb.sum())
    offM = np.r_[0, np.cumsum(ja + jb)[:-1]]

    prep.counts = np.bincount(batch, minlength=N_GRAPHS).astype(np.float32)

    for k in range(N_CORES):
        sel = e_core == k
        p_k = e_pos[sel]
        j_k = e_j[sel]
        a_k = src_half_a[sel]
        g_k = e_gsrc[sel]
        b_k = p_k // 128
        pp_k = p_k % 128

        idxA = np.zeros(SA, dtype=np.int64)
        idxB = np.zeros(SB, dtype=np.int64)
        mask = np.full((128, JTOT), -1e30, dtype=np.float32)

        sa = a_k
        iA = offA[b_k[sa]] + j_k[sa] * 128 + pp_k[sa]
        idxA[iA] = g_k[sa]
        mask[pp_k[sa], offM[b_k[sa]] + j_k[sa]] = 0.0
        sb_ = ~a_k
        iB = offB[b_k[sb_]] + j_k[sb_] * 128 + pp_k[sb_]
        idxB[iB] = g_k[sb_] - HG
        mask[pp_k[sb_], offM[b_k[sb_]] + ja[b_k[sb_]] + j_k[sb_]] = 0.0

        prep.idxa_w.append(_wrap_idxs(idxA))
        prep.idxb_w.append(_wrap_idxs(idxB))
        prep.mask_u.append(mask)

        oh = np.zeros((128, NB * N_GRAPHS), dtype=np.float32)
        nat = perms[k]
        bvals = batch[k * NPC + nat]
        ppos = np.arange(NPC)
        oh[ppos % 128, (ppos // 128) * N_GRAPHS + bvals] = 1.0
        prep.onehot.append(oh)

    return prep


def fold_weights(W1, att_src1, att_dst1, W2, att_src2, att_dst2):
    W1e = np.zeros((IN_CH, D1 + 2 * HEADS), dtype=np.float32)
    W1e[:, :D1] = W1
    for h in range(HEADS):
        W1e[:, D1 + h] = W1[:, h * HID : (h + 1) * HID] @ att_src1[h]
        W1e[:, D1 + HEADS + h] = W1[:, h * HID : (h + 1) * HID] @ att_dst1[h]
    W2e = np.zeros((D1, D2 + 2), dtype=np.float32)
    W2e[:, :D2] = W2
    W2e[:, D2] = W2 @ att_src2[0]
    W2e[:, D2 + 1] = W2 @ att_dst2[0]
    return W1e, W2e


BF16 = mybir.dt.bfloat16
ROW1 = 128  # bf16 elems per layer-1 table row (256B): [xl1 x64 | a_s1 f32 x2 | pad]
ROW2 = 128  # bf16 elems per layer-2 table row (256B): [xl2 x128]


def _do_gathers(nc, g, table, half_rows, d, ja_b, jb_b, J, idxa_t, idxb_t, qa=0, qb=0):
    # split each half's gather in two and fan across all 4 SWDGE queues to
    # deepen DMA in-flight parallelism (random 256B reads are latency-bound)
    parts = []
    nsplit = 3
    for (w, tab, it, base) in [
        (ja_b, table[0:half_rows, :], idxa_t, 0),
        (jb_b, table[half_rows : 2 * half_rows, :], idxb_t, ja_b),
    ]:
        cuts = [w * i // nsplit for i in range(nsplit + 1)]
        for c0, c1 in zip(cuts[:-1], cuts[1:]):
            if c1 > c0:
                parts.append((base + c0, base + c1, tab, it[:, 8 * c0 : 8 * c1]))
    for i, (j0, j1, tab, idxs) in enumerate(parts):
        nj = j1 - j0
        nc.gpsimd.dma_gather(
            g[:, j0:j1, :], tab, idxs, nj * 128, nj * 128, d,
            single_packet=False, queue_num=(qa + i) % 4,
        )


def _edge_block(nc, gpool, pool, table, half_rows, layer, ja_b, jb_b,
               idxa_dram, idxb_dram, mask_dram, ad_ap, att_sb, bias_sb, neg,
                qa=0, qb=0):
    """One block of the edge pipeline (bf16 tables).

    layer=1: row = [xl1 bf16 x64 | a_s1 as f32 x2 | pad], d=64, H=2
    layer=2: row = [xl2 bf16 x128], d=128, H=1 (a_s2 via DVE mul+reduce)
    Returns SBUF tile [128, d] f32 (post-ELU).
    """
    J = ja_b + jb_b
    d = 64 if layer == 1 else 128
    H = 2 if layer == 1 else 1
    DH = d // H
    row = ROW1 if layer == 1 else ROW2
    idxa_t = pool.tile([128, 8 * ja_b], I16, tag="idxa")
    idxb_t = pool.tile([128, 8 * jb_b], I16, tag="idxb")
    mask_t = pool.tile([128, J], F32, tag="mask")
    nc.sync.dma_start(idxa_t[:], idxa_dram)
    nc.sync.dma_start(idxb_t[:], idxb_dram)
    nc.sync.dma_start(mask_t[:], mask_dram)

    g = gpool.tile([128, J, row], BF16, tag="g")
    _do_gathers(nc, g, table, half_rows, row, ja_b, jb_b, J, idxa_t, idxb_t, qa, qb)

    xl = g[:, :, 0:d]  # [128, J, d] bf16
    xl4 = xl.rearrange("p j (h c) -> p j h c", h=H)
    # per-edge a_s
    if layer == 1:
        a_s = g[:, :, d : d + 2 * H].bitcast(F32)  # [128, J, H] f32
    else:
        tmp = gpool.tile([128, J, d], BF16, tag="tmp")
        nc.vector.tensor_tensor(
            tmp[:, :, :].rearrange("p j (h c) -> p j h c", h=H),
            xl4,
            att_sb.rearrange("p (h c) -> p h c", h=H).unsqueeze(1).broadcast_to([128, J, H, DH]),
            OP.mult,
        )
        a_s = pool.tile([128, J, H], F32, tag="a_s")
        nc.vector.tensor_reduce(
            a_s[:, :, :], tmp[:, :, :].rearrange("p j (h c) -> p j h c", h=H),
            mybir.AxisListType.X, OP.add,
        )
    s = pool.tile([128, J, H], F32, tag="s")
    for h in range(H):
        nc.vector.scalar_tensor_tensor(
            s[:, :, h], a_s[:, :, h], ad_ap[:, h : h + 1], mask_t[:, :], OP.add, OP.add
        )
    lr = pool.tile([128, J, H], F32, tag="lr")
    nc.vector.scalar_tensor_tensor(lr[:, :, :], s[:, :, :], neg, s[:, :, :], OP.mult, OP.max)
    ex = pool.tile([128, J, H], F32, tag="ex")
    denom = pool.tile([128, H], F32, tag="denom")
    for h in range(H):
        nc.scalar.activation(ex[:, :, h], lr[:, :, h], AF.Exp, accum_out=denom[:, h : h + 1])
    # msg = xl * ex (bf16 out, in-place over xl)
    nc.vector.tensor_tensor(xl4, xl4, ex[:, :, :].unsqueeze(3).broadcast_to([128, J, H, DH]), OP.mult)
    osum = pool.tile([128, d], F32, tag="osum")
    nc.vector.tensor_reduce(
        osum[:, :].rearrange("p (h c) -> p h c", h=H),
        xl.rearrange("p j (h c) -> p h c j", h=H),
        mybir.AxisListType.X, OP.add,
    )
    dcl = pool.tile([128, H], F32, tag="dcl")
    nc.vector.tensor_scalar(dcl[:], denom[:], 1e-30, None, OP.max)
    rden = pool.tile([128, H], F32, tag="rden")
    nc.vector.reciprocal(rden[:], dcl[:])
    pre = pool.tile([128, d], F32, tag="pre")
    for h in range(H):
        nc.vector.scalar_tensor_tensor(
            pre[:, h * DH : (h + 1) * DH], osum[:, h * DH : (h + 1) * DH],
            rden[:, h : h + 1], bias_sb[:, h * DH : (h + 1) * DH], OP.mult, OP.add,
        )
    e1 = pool.tile([128, d], F32, tag="e1")
    nc.scalar.activation(e1[:], pre[:], AF.Exp)
    t2 = pool.tile([128, d], F32, tag="t2")
    nc.vector.tensor_scalar(t2[:], e1[:], 1.0, 0.0, OP.subtract, OP.min)
    res = pool.tile([128, d], F32, tag="res")
    nc.vector.tensor_tensor(res[:], pre[:], t2[:], OP.max)
    return res


def build_kernel(ja: list, jb: list, nq: int = 4, repeat: int = 1, sim: bool = False):
    C, H, G, IN = N_CORES, HEADS, N_GRAPHS, IN_CH
    SA = sum(ja) * 128
    SB = sum(jb) * 128
    JTOT = sum(ja) + sum(jb)
    offA = np.r_[0, np.cumsum(ja)[:-1]] * 128
    offB = np.r_[0, np.cumsum(jb)[:-1]] * 128
    offM = np.r_[0, np.cumsum(np.array(ja) + np.array(jb))[:-1]]

    nc = bacc.Bacc("TRN2", target_bir_lowering=False, debug=False, num_devices=C,
                   num_swdge_queues=nq)

    xT = nc.dram_tensor("xT", [IN, NPP], F32, kind="ExternalInput")
    w1e = nc.dram_tensor("w1e", [IN, D1 + 2 * H], F32, kind="ExternalInput")
    w2e = nc.dram_tensor("w2e", [D1, D2 + 2], F32, kind="ExternalInput")
    b1r = nc.dram_tensor("b1r", [128, D1], F32, kind="ExternalInput")
    b2r = nc.dram_tensor("b2r", [128, D2], F32, kind="ExternalInput")
    att2 = nc.dram_tensor("att2", [128, D2], F32, kind="ExternalInput")
    ident = nc.dram_tensor("ident", [128, 128], F32, kind="ExternalInput")
    idxa_d = nc.dram_tensor("idxa", [128, SA // 16], I16, kind="ExternalInput")
    idxb_d = nc.dram_tensor("idxb", [128, SB // 16], I16, kind="ExternalInput")
    mask_d = nc.dram_tensor("mask", [128, JTOT], F32, kind="ExternalInput")
    oneh_d = nc.dram_tensor("onehot", [128, NB * G], F32, kind="ExternalInput")
    out_d = nc.dram_tensor("pool_out", [G, D2], F32, kind="ExternalOutput")

    with tile.TileContext(nc) as tc:
        nc.gpsimd.load_library(library_config.mlp)
        with (
            tc.tile_pool(name="const", bufs=1) as cpool,
            tc.tile_pool(name="gtiles", bufs=5) as gpool_g,
            tc.tile_pool(name="ttiles", bufs=3) as gpool_t,
            tc.tile_pool(name="work", bufs=3) as pool,
            tc.tile_pool(name="psum", bufs=2, space="PSUM") as psum,
            tc.tile_pool(name="poolacc", bufs=1, space="PSUM") as psacc,
            tc.tile_pool(name="dram", bufs=1, space="DRAM") as dpool,
        ):
            class GPool:
                def tile(self, shape, dt_, tag):
                    p = gpool_g if tag == "g" else gpool_t
                    return p.tile(shape, dt_, tag=tag, name=tag)

            gpool = GPool()
            w1e_sb = cpool.tile([IN, D1 + 2 * H], F32)
            w2e_sb = cpool.tile([D1, D2 + 2], F32)
            b1_sb = cpool.tile([128, D1], F32)
            b2_sb = cpool.tile([128, D2], F32)
            att2_sb = cpool.tile([128, D2], BF16)
            att2_f32 = cpool.tile([128, D2], F32)
            id_sb = cpool.tile([128, 128], F32)

            for t, srcd in [
                (w1e_sb, w1e), (w2e_sb, w2e), (b1_sb, b1r), (b2_sb, b2r),
                (att2_f32, att2), (id_sb, ident),
            ]:
                nc.sync.dma_start(t[:], srcd[:])
            nc.vector.tensor_copy(att2_sb[:], att2_f32[:])

            for _rep in range(repeat):
                ad1_sb = cpool.tile([128, NB, H], F32)
                ad2_sb = cpool.tile([128, NB], F32)
                hT_sb = cpool.tile([D1, NPP], F32)

                # P1: layer-1 table (bf16 rows with packed f32 a_s)
                xl1_own = dpool.tile([NPP, ROW1], BF16)
                table1 = nc.dram_tensor(
                    f"table1_sh_{_rep}", [NG, ROW1], BF16,
                    kind="Internal", addr_space="Shared",
                ).ap()
                for c in range(NB):
                    xt_t = pool.tile([IN, 128], F32, tag="xt")
                    nc.sync.dma_start(xt_t[:], xT[:, c * 128 : (c + 1) * 128])
                    ps = psum.tile([128, D1 + 2 * H], F32, tag="mm1")
                    nc.tensor.matmul(ps[:], xt_t[:], w1e_sb[:], start=True, stop=True)
                    rowt = pool.tile([128, ROW1], BF16, tag="row1")
                    nc.vector.memset(rowt[:, D1 + 2 * H : ROW1], 0.0)
                    nc.vector.tensor_copy(rowt[:, 0:D1], ps[:, 0:D1])
                    nc.vector.tensor_copy(
                        rowt[:, D1 : D1 + 2 * H].bitcast(F32), ps[:, D1 : D1 + H]
                    )
                    nc.sync.dma_start(xl1_own[c * 128 : (c + 1) * 128, :], rowt[:])
                    nc.vector.tensor_copy(ad1_sb[:, c, :], ps[:, D1 + H : D1 + 2 * H])
                if sim:
                    nc.sync.dma_start(table1[0:NPP, :], xl1_own[:])
                else:
                    nc.gpsimd.collective_compute(
                        "AllGather", OP.bypass, replica_groups=[list(range(C))],
                        ins=[xl1_own.opt()], outs=[table1],
                    )

                # P2: layer-1 edges
                for b in range(NB):
                    res = _edge_block(
                        nc, gpool, pool, table1, HG, 1, ja[b], jb[b],
                        idxa_d[:, offA[b] // 16 : (offA[b] + 128 * ja[b]) // 16],
                        idxb_d[:, offB[b] // 16 : (offB[b] + 128 * jb[b]) // 16],
                        mask_d[:, offM[b] : offM[b] + ja[b] + jb[b]],
                        ad1_sb[:, b, :], None, b1_sb[:], NEG_SLOPE,
                        qa=(2 * b) % nq, qb=(2 * b + 1) % nq,
                    )
                    pst = psum.tile([D1, 128], F32, tag="tps")
                    nc.tensor.transpose(pst[:], res[:], id_sb[:])
                    nc.vector.tensor_copy(hT_sb[:, b * 128 : (b + 1) * 128], pst[:])

                # P3: layer-2 table (pure bf16 rows)
                xl2_own = dpool.tile([NPP, ROW2], BF16)
                table2 = nc.dram_tensor(
                    f"table2_sh_{_rep}", [NG, ROW2], BF16,
                    kind="Internal", addr_space="Shared",
                ).ap()
                for c in range(NB):
                    ps = psum.tile([128, D2 + 2], F32, tag="mm2")
                    nc.tensor.matmul(
                        ps[:], hT_sb[:, c * 128 : (c + 1) * 128], w2e_sb[:],
                        start=True, stop=True,
                    )
                    rowt = pool.tile([128, ROW2], BF16, tag="row2")
                    nc.vector.tensor_copy(rowt[:, 0:D2], ps[:, 0:D2])
                    nc.sync.dma_start(xl2_own[c * 128 : (c + 1) * 128, :], rowt[:])
                    nc.vector.tensor_copy(ad2_sb[:, c : c + 1], ps[:, D2 + 1 : D2 + 2])
                if sim:
                    nc.sync.dma_start(table2[0:NPP, :], xl2_own[:])
                else:
                    nc.gpsimd.collective_compute(
                        "AllGather", OP.bypass, replica_groups=[list(range(C))],
                        ins=[xl2_own.opt()], outs=[table2],
                    )

                # P4: layer-2 edges + pool
                pacc = psacc.tile([G, D2], F32)
                for b in range(NB):
                    res = _edge_block(
                        nc, gpool, pool, table2, HG, 2, ja[b], jb[b],
                        idxa_d[:, offA[b] // 16 : (offA[b] + 128 * ja[b]) // 16],
                        idxb_d[:, offB[b] // 16 : (offB[b] + 128 * jb[b]) // 16],
                        mask_d[:, offM[b] : offM[b] + ja[b] + jb[b]],
                        ad2_sb[:, b : b + 1], att2_sb[:], b2_sb[:], NEG_SLOPE,
                        qa=(2 * b) % nq, qb=(2 * b + 1) % nq,
                    )
                    oh_t = pool.tile([128, G], F32, tag="oh", name="oh")
                    nc.sync.dma_start(oh_t[:], oneh_d[:, b * G : (b + 1) * G])
                    nc.tensor.matmul(
                        pacc[:], oh_t[:], res[:],
                        start=(b == 0), stop=(b == NB - 1),
                    )
                out_sb = pool.tile([G, D2], F32, tag="outsb")
                nc.vector.tensor_copy(out_sb[:], pacc[:])
                nc.sync.dma_start(out_d[:], out_sb[:])

    nc.compile()
    return nc



def _make_in_maps(x, W1e, W2e, b1, b2, as1f, as2f, prep: Prep):
    ident = np.eye(128, dtype=np.float32)
    b1r = np.tile(np.asarray(b1, np.float32).reshape(1, -1), (128, 1))
    b2r = np.tile(np.asarray(b2, np.float32).reshape(1, -1), (128, 1))
    att2 = np.tile(as2f.reshape(1, -1).astype(np.float32), (128, 1))
    in_maps = []
    for k in range(N_CORES):
        xk = np.zeros((IN_CH, NPP), dtype=np.float32)
        xk[:, :NPC] = x[k * NPC + prep.perms[k]].T
        in_maps.append(
            {
                "xT": xk, "w1e": W1e, "w2e": W2e, "b1r": b1r, "b2r": b2r,
                "att2": att2, "ident": ident,
                "idxa": prep.idxa_w[k], "idxb": prep.idxb_w[k],
                "mask": prep.mask_u[k], "onehot": prep.onehot[k],
            }
        )
    return in_maps


_CACHE = {}


def kernel(x, edge_index, batch, W1, att_src1, att_dst1, b1, W2, att_src2, att_dst2, b2):
    x = np.asarray(x, dtype=np.float32)
    edge_index = np.asarray(edge_index, dtype=np.int64)
    batch = np.asarray(batch, dtype=np.int64)
    W1 = np.asarray(W1, dtype=np.float32)
    W2 = np.asarray(W2, dtype=np.float32)
    att_src1 = np.asarray(att_src1, dtype=np.float32)
    att_dst1 = np.asarray(att_dst1, dtype=np.float32)
    att_src2 = np.asarray(att_src2, dtype=np.float32)
    att_dst2 = np.asarray(att_dst2, dtype=np.float32)

    prep = preprocess(edge_index, batch)
    W1e, W2e = fold_weights(W1, att_src1, att_dst1, W2, att_src2, att_dst2)

    key = (tuple(prep.ja), tuple(prep.jb))
    if key not in _CACHE:
        _CACHE[key] = build_kernel(prep.ja, prep.jb)
    nc = _CACHE[key]

    in_maps = _make_in_maps(
        x, W1e, W2e, b1, b2, att_src1.reshape(-1), att_src2.reshape(-1), prep
    )
    res = None
    last_err = None
    for attempt in range(4):
        try:
            res = run_bass_kernel_spmd(
                nc, in_maps, core_ids=list(range(N_CORES)), trace=False
            )
            break
        except Exception as e:  # device flake: reset backends and retry
            last_err = e
            import time as _time

            _time.sleep(8.0 * (attempt + 1))
            try:
                import jax as _jax

                _jax.clear_caches()
                _jax.extend.backend.clear_backends()
            except Exception:
                pass
    if res is None:
        raise last_err

    total = np.zeros((N_GRAPHS, OUT_CH), np.float64)
    for k in range(N_CORES):
        total += res.results[k]["pool_out"]
    out = total / np.maximum(prep.counts, 1.0)[:, None]
    return out.astype(np.float32)

